# revision 32
# baseline (speedup 1.0000x reference)
"""GroupedQueryAttention TRN2 Bass kernel, 8-way (kv-group, batch) sharded.

B=2, S=2048, E=2048, H=16 q-heads, KVH=4 kv-heads, HD=128.
Core d = (g, b) with g = d//2 (kv group), b = d%2 (batch): it computes the
4 q-heads of group g and kv-head g for batch b only.  Unlike plain head
sharding this has ZERO redundant projection work, and every core touches
only half of x (its batch) and emits a [S, E] partial (contraction over its
512 head dims); the host sums 4 partials per batch.

Precision (error budget max|diff|/mean|expected| < 2e-2; measured fp16
rounding contributions: x/w 1.5e-2, v 1e-2, sumexp-chain 1e-2, q/k 7e-3 —
so the datapath stays fp32r end to end, like the 1.1e-2 baseline):
  fp32r everywhere on the PE; fp32 PSUM; fp16 only for the 0/1 masks
  (exact) and the output partials (adds ~6e-3, well inside budget, and
  halves the write traffic).

Layout (all matmuls natural):
  phase 1, per 512-token tile, two passes to fit PSUM (A: q0,q1,k / B:
    q2,q3,v): qT/kT/vT[hd, tok] = W.T @ xT with x streamed through a
    2-deep tile pool, RoPE on the PSUM->SBUF epilogue (1/sqrt(HD) folded
    into wq; the sin-term multiply rides the otherwise-idle gpsimd);
    v transposed to natural [tok, hd] via the PE.
  attention per (q-tile, head), flash-style over PAIRS of 128-wide key
    chunks: scoresT[kt, qt] = kT_chunk.T @ qT_tile -> exp -> ctxT[hd, qt] +=
    v_chunk.T @ P.  Causal: only chunks up to the diagonal; ctx/sump
    matmuls skip the fully-masked column prefix via `delta`; diagonal
    chunks are exp'd over the FULL tile in one Act op (the masked-prefix
    garbage is stale finite scores; the banks are zeroed once at phase-2
    open) and zeroed by the 0/1 mask multiply.  sumexp[128bcast, qt]:
    non-diagonal pairs are pre-added on the DVE (padd) then one
    ones-matmul per pair; diagonal chunks go in via delta-sliced
    ones-matmuls.  All consumers of a pair's pexp are emitted one pair
    late so the PE never queues a matmul behind a fresh Act/DVE dep.
  out_proj per 128-token chunk of the PREVIOUS q-tile (emitted after head 0
    of the next q-tile so its matmuls fill the normalize-tail bubble):
    out[tok, e] = sum_h ctx_h.T @ wo_h, drained PSUM->SBUF (fp16)
    alternating DVE/Act, DMA'd per [128, 512] block from alternating
    queues.
"""
import sys
sys.path.insert(0, '/opt/trn_rl_repo')

import numpy as np
from contextlib import ExitStack

import concourse.bass as bass
import concourse.bacc as bacc
import concourse.tile as tile
from concourse import mybir
from concourse.bass_utils import run_bass_kernel_spmd
from concourse.alu_op_type import AluOpType

F32 = mybir.dt.float32
F32R = mybir.dt.float32r
F16 = mybir.dt.float16
EXP = mybir.ActivationFunctionType.Exp

B, S, E = 2, 2048, 2048
H, KVH, HD = 16, 4, 128
NCORES = 8
NT = 512                   # token tile (matmul free dim)
NTT = S // NT              # 4 token tiles per core
KC = E // 128              # 16 contraction chunks for projections
KB = S // 128              # 16 key chunks per core
NQH = 4                    # q heads per core
ROPE_BASE = 10000.0

_CACHE = {}


def _emit(nc, tc, ctx):
    xT_d = nc.declare_dram_parameter("xT", [E, S], F32R, isOutput=False)
    wq_d = nc.declare_dram_parameter("wq", [E, NQH * HD], F32R, isOutput=False)
    wk_d = nc.declare_dram_parameter("wk", [E, HD], F32R, isOutput=False)
    wv_d = nc.declare_dram_parameter("wv", [E, HD], F32R, isOutput=False)
    wo_d = nc.declare_dram_parameter("wo", [NQH * HD, E], F32R, isOutput=False)
    cos_d = nc.declare_dram_parameter("cos", [HD, S], F32, isOutput=False)
    sinm_d = nc.declare_dram_parameter("sinm", [HD, S], F32, isOutput=False)
    masks_d = nc.declare_dram_parameter("masks", [4, 128, NT], F16, isOutput=False)
    ident_d = nc.declare_dram_parameter("ident", [128, 128], F32R, isOutput=False)
    onec_d = nc.declare_dram_parameter("onec", [128, 128], F32R, isOutput=False)
    out_d = nc.declare_dram_parameter("out", [S, E], F16, isOutput=True)

    persist = ctx.enter_context(tc.tile_pool(name="persist", bufs=1))
    qT = [persist.tile([HD, S], F32R, name=f"qT{i}") for i in range(NQH)]
    kT = persist.tile([HD, S], F32R)
    v_sb = persist.tile([128, KB, HD], F32R)    # v natural: [tok%128, blk, hd]
    wq_s = persist.tile([128, KC, NQH * HD], F32R)
    wk_s = persist.tile([128, KC, HD], F32R)
    wv_s = persist.tile([128, KC, HD], F32R)
    cos_s = persist.tile([HD, S], F32)
    sinm_s = persist.tile([HD, S], F32)
    masks_s = persist.tile([128, 4, NT], F16)
    ident = persist.tile([128, 128], F32R)
    ones_col = persist.tile([128, 128], F32R)

    wqv = wq_d.rearrange("(k p) m -> p k m", p=128)
    wkv = wk_d.rearrange("(k p) m -> p k m", p=128)
    wvv = wv_d.rearrange("(k p) m -> p k m", p=128)
    xT_view = xT_d.rearrange("(k p) t -> p k t", p=128)
    wov = wo_d.rearrange("(h p) e -> p h e", p=128)

    # ---------------- phase 1: projections + RoPE + v transpose ----------------
    with ExitStack() as p1:
        xpool = p1.enter_context(tc.tile_pool(name="xpool", bufs=2))
        rope = p1.enter_context(tc.tile_pool(name="rope", bufs=4))
        vstage = p1.enter_context(tc.tile_pool(name="vstage", bufs=2))
        psA = p1.enter_context(tc.tile_pool(name="psA", bufs=1, space="PSUM"))
        psB = p1.enter_context(tc.tile_pool(name="psB", bufs=1, space="PSUM"))
        pst = p1.enter_context(tc.tile_pool(name="pst", bufs=2, space="PSUM"))

        def load_xtile(tt):
            xt = xpool.tile([128, KC, NT], F32R, tag="x", name="xt")
            t0 = tt * NT
            nc.sync.dma_start(xt[:, 0:8, :], xT_view[:, 0:8, t0:t0 + NT])
            nc.gpsimd.dma_start(xt[:, 8:16, :], xT_view[:, 8:16, t0:t0 + NT])
            return xt

        # first matmul needs only wq chunk 0 and the first x chunk: tiny
        # loads first, then everything else in rough consumption order
        xt0 = xpool.tile([128, KC, NT], F32R, tag="x", name="xt")
        nc.sync.dma_start(xt0[:, 0, :], xT_view[:, 0, 0:NT])
        nc.gpsimd.dma_start(wq_s[:, 0, :], wqv[:, 0, :])
        nc.sync.dma_start(wk_s[:, 0:4, :], wkv[:, 0:4, :])
        nc.gpsimd.dma_start(xt0[:, 1:8, :], xT_view[:, 1:8, 0:NT])
        nc.sync.dma_start(xt0[:, 8:16, :], xT_view[:, 8:16, 0:NT])
        nc.gpsimd.dma_start(wq_s[:, 1:4, :], wqv[:, 1:4, :])
        nc.gpsimd.dma_start(wv_s[:, 0:4, :], wvv[:, 0:4, :])
        for kq in range(1, 4):
            ks = slice(4 * kq, 4 * kq + 4)
            (nc.gpsimd if kq % 2 else nc.sync).dma_start(wq_s[:, ks, :], wqv[:, ks, :])
        nc.sync.dma_start(wk_s[:, 4:16, :], wkv[:, 4:16, :])
        nc.gpsimd.dma_start(wv_s[:, 4:16, :], wvv[:, 4:16, :])
        nc.scalar.dma_start(cos_s[:], cos_d[:, :])
        nc.scalar.dma_start(sinm_s[:], sinm_d[:, :])
        nc.scalar.dma_start(masks_s[:], masks_d.rearrange("m p j -> p m j"))
        nc.scalar.dma_start(ident[:], ident_d[:, :])
        nc.scalar.dma_start(ones_col[:], onec_d[:, :])
        xtiles = [xt0, load_xtile(1)]

        def rope_drain(psum, dest, t0):
            # dest = psum*cos + swap_halves(psum)*sinm, computed in fp32
            # (sinm has -sin in the top half).  Phase 1 is DMA-bound, so the
            # DVE has headroom for all three multiplies; keeping gpsimd out
            # of the sw ring matters (its tensor ops run at ~2x DVE latency
            # and would gate the PSUM-bank release via the pool rotation).
            sw = rope.tile([HD, NT], F32, tag="sw")
            nc.scalar.copy(sw[0:64, :], psum[64:128, :])
            nc.scalar.copy(sw[64:128, :], psum[0:64, :])
            nc.vector.tensor_tensor(sw[:], sw[:], sinm_s[:, t0:t0 + NT],
                                    AluOpType.mult)
            d = dest[:, t0:t0 + NT]
            nc.vector.tensor_tensor(d, psum[:], cos_s[:, t0:t0 + NT],
                                    AluOpType.mult)
            nc.vector.tensor_tensor(d, d, sw[:], AluOpType.add)

        for tt in range(NTT):
            t0 = tt * NT
            x_sb = xtiles.pop(0)
            if tt + 2 < NTT:
                xtiles.append(load_xtile(tt + 2))
            # pass A: q0, q1, k
            pq0 = psA.tile([HD, NT], F32, tag="q0")
            pq1 = psA.tile([HD, NT], F32, tag="q1")
            pk = psA.tile([HD, NT], F32, tag="k")
            for k in range(KC):
                xck = x_sb[:, k, :]
                st, sp = (k == 0), (k == KC - 1)
                nc.tensor.matmul(pq0[:], wq_s[:, k, 0:HD], xck, start=st, stop=sp)
                nc.tensor.matmul(pq1[:], wq_s[:, k, HD:2 * HD], xck, start=st, stop=sp)
                nc.tensor.matmul(pk[:], wk_s[:, k, :], xck, start=st, stop=sp)
            rope_drain(pq0, qT[0], t0)
            rope_drain(pq1, qT[1], t0)
            rope_drain(pk, kT, t0)
            # pass B: q2, q3, v
            pq2 = psB.tile([HD, NT], F32, tag="q2")
            pq3 = psB.tile([HD, NT], F32, tag="q3")
            pv = psB.tile([HD, NT], F32, tag="v")
            for k in range(KC):
                xck = x_sb[:, k, :]
                st, sp = (k == 0), (k == KC - 1)
                nc.tensor.matmul(pq2[:], wq_s[:, k, 2 * HD:3 * HD], xck, start=st, stop=sp)
                nc.tensor.matmul(pq3[:], wq_s[:, k, 3 * HD:4 * HD], xck, start=st, stop=sp)
                nc.tensor.matmul(pv[:], wv_s[:, k, :], xck, start=st, stop=sp)
            rope_drain(pq2, qT[2], t0)
            rope_drain(pq3, qT[3], t0)
            vT_s = vstage.tile([HD, NT], F32R, tag="vT")
            nc.scalar.copy(vT_s[:], pv[:])
            for c in range(NT // 128):
                tp = pst.tile([128, 128], F32R, tag="tp")
                nc.tensor.matmul(tp[:], vT_s[:, c * 128:(c + 1) * 128], ident[:],
                                 is_transpose=True)
                nc.vector.tensor_copy(v_sb[:, tt * 4 + c, :], tp[:])

    # ---------- phase 2: attention + out_proj, interleaved per q-tile ----------
    with ExitStack() as p2:
        wopool = p2.enter_context(tc.tile_pool(name="wopool", bufs=1))
        ppool = p2.enter_context(tc.tile_pool(name="ppool", bufs=5))
        paddp = p2.enter_context(tc.tile_pool(name="paddp", bufs=3))
        accp = p2.enter_context(tc.tile_pool(name="accp", bufs=2))
        bcsp = p2.enter_context(tc.tile_pool(name="bcsp", bufs=2))
        cxp = p2.enter_context(tc.tile_pool(name="cxp", bufs=2))
        obp = p2.enter_context(tc.tile_pool(name="obp", bufs=4))
        # PSUM budget (8 banks): scores pairs 2x2 | ctx 1 | sumexp 1 | out 2
        pss = p2.enter_context(tc.tile_pool(name="pss", bufs=2, space="PSUM"))
        psc = p2.enter_context(tc.tile_pool(name="psc", bufs=1, space="PSUM"))
        psn = p2.enter_context(tc.tile_pool(name="psn", bufs=1, space="PSUM"))
        pso = p2.enter_context(tc.tile_pool(name="pso", bufs=2, space="PSUM"))

        wo_s = wopool.tile([128, NQH, E], F32R)  # [hd%128, head, e]
        nc.scalar.dma_start(wo_s[:], wov[:, :, :])

        # zero the scores banks once: the full-range diagonal exp below may
        # read whatever a previous NEFF left in PSUM, which must stay finite
        # under exp
        for _ in range(2):
            z = pss.tile([128, 2, NT], F32, tag="s", name="z")
            nc.vector.memset(z[:], 0.0)

        def out_proj_steps(qt, cx):
            # generator: one (token-chunk, e-chunk) block per step, so the
            # blocks can be zipped between attention pairs — their matmuls
            # fill the PE while the Act engine works through the exps
            for tc4 in range(NT // 128):
                tch = qt * (NT // 128) + tc4
                tsl = slice(tch * 128, (tch + 1) * 128)
                for ech in range(E // NT):
                    esl = slice(ech * NT, (ech + 1) * NT)
                    op = pso.tile([128, NT], F32, tag="o")
                    for h in range(NQH):
                        nc.tensor.matmul(op[:], cx[h][:, tc4 * 128:(tc4 + 1) * 128],
                                         wo_s[:, h, esl],
                                         start=(h == 0), stop=(h == NQH - 1))
                    ob = obp.tile([128, NT], F16, tag="ob")
                    if ech % 2 == 0:
                        nc.vector.tensor_copy(ob[:], op[:])
                    else:
                        nc.scalar.copy(ob[:], op[:])
                    (nc.sync if ech % 2 == 0 else nc.gpsimd).dma_start(
                        out_d[tsl, esl], ob[:])
                    yield

        DEPTH = 3  # deferral depth in pairs: PE always has ~3us queued
        pending = None
        for qt in range(NTT):
            npairs = 2 * (qt + 1)
            nk = 4 * (qt + 1)
            q_sl = slice(qt * NT, (qt + 1) * NT)

            def delta(kc):
                # fully-masked column prefix of a diagonal chunk
                return (kc - 4 * qt) * 128 if kc >= 4 * qt else 0

            cx = [cxp.tile([HD, NT], F32R, tag=f"cx{i}", name=f"cx{i}")
                  for i in range(NQH)]
            # per-head PSUM state, created lazily at first deferred flush
            hstate = {}

            def flush(item):
                # emit the ctx/sump matmuls for a pair DEPTH pairs after its
                # scores/exp were issued, so the PE never reaches a matmul
                # whose Act/DVE producer hasn't finished
                h, j, diag, pexp, acc = item
                if h not in hstate:
                    ctxp = psc.tile([HD, NT], F32, tag="ctx", name="ctxp")
                    sump = psn.tile([128, NT], F32, tag="sum", name="sump")
                    hstate[h] = (ctxp, sump)
                ctxp, sump = hstate[h]
                if diag and j == 2 * qt and qt > 0:
                    # fold the DVE-accumulated non-diagonal pexp sum in
                    nc.tensor.matmul(sump[:], ones_col[:], acc[:],
                                     start=True, stop=False)
                for half in (0, 1):
                    kc = 2 * j + half
                    dl = delta(kc)
                    st, sp_ = (kc == 0), (kc == nk - 1)
                    nc.tensor.matmul(
                        ctxp[:, dl:], v_sb[:, kc, :],
                        pexp[:, half, dl:], start=st, stop=sp_)
                    if diag:
                        nc.tensor.matmul(sump[:, dl:], ones_col[:],
                                         pexp[:, half, dl:],
                                         start=(qt == 0 and kc == 0),
                                         stop=sp_)
                if j == npairs - 1:
                    # normalize straight out of PSUM: recip of the broadcast
                    # sumexp (all-ones stationary), one fused multiply
                    bcs = bcsp.tile([128, NT], F32, tag="bcs")
                    nc.vector.reciprocal_approx_fast(bcs[:], sump[:])
                    nc.vector.tensor_tensor(cx[h][:], ctxp[:], bcs[:],
                                            AluOpType.mult)
                    return h
                return None

            dq = []
            for h in range(NQH):
                acc = None
                prev_padd = None
                for j in range(npairs):
                    diag = j >= 2 * qt
                    sp2 = pss.tile([128, 2, NT], F32, tag="s")
                    for half in (0, 1):
                        kc = 2 * j + half
                        dl = delta(kc)
                        nc.tensor.matmul(
                            sp2[:, half, dl:],
                            kT[:, kc * 128:(kc + 1) * 128],
                            qT[h][:, q_sl][:, dl:])
                    pexp = ppool.tile([128, 2, NT], F32R, tag="p")
                    # one full-range exp per pair; for diagonal pairs the
                    # masked-prefix region holds exp(stale scores) — finite
                    # — and is zeroed by the mask multiply below
                    nc.scalar.activation(pexp[:], sp2[:], EXP)
                    if diag:
                        for half in (0, 1):
                            kc = 2 * j + half
                            nc.vector.tensor_tensor(
                                pexp[:, half, :], pexp[:, half, :],
                                masks_s[:, kc - 4 * qt, :], AluOpType.mult)
                    else:
                        # DVE tree: pair-add, then fold into the running acc
                        # (one ones-matmul per head instead of one per pair)
                        padd = paddp.tile([128, NT], F32R, tag="padd")
                        nc.vector.tensor_tensor(padd[:], pexp[:, 0, :],
                                                pexp[:, 1, :], AluOpType.add)
                        if prev_padd is None:
                            prev_padd = padd
                        elif acc is None:
                            acc = accp.tile([128, NT], F32R, tag="acc",
                                            name="acc")
                            nc.vector.tensor_tensor(acc[:], prev_padd[:],
                                                    padd[:], AluOpType.add)
                        else:
                            nc.vector.tensor_tensor(acc[:], acc[:], padd[:],
                                                    AluOpType.add)
                    dq.append((h, j, diag, pexp, acc))
                    if len(dq) > DEPTH:
                        done_h = flush(dq.pop(0))
                        if done_h == 0 and pending is not None:
                            # head 0 normalized: the previous q-tile's
                            # out_proj runs here, while this q-tile's
                            # remaining exps stream on the Act engine
                            for _ in pending:
                                pass
                            pending = None
            while dq:
                flush(dq.pop(0))
            if pending is not None:  # qt0 spill (fewer pairs than DEPTH)
                for _ in pending:
                    pass
            pending = out_proj_steps(qt, cx)
        for _ in pending:
            pass


def _build():
    if "nc" in _CACHE:
        return _CACHE["nc"]
    nc = bacc.Bacc("TRN2", target_bir_lowering=False, debug=False,
                   num_devices=NCORES)
    with tile.TileContext(nc) as tc:
        with nc.allow_low_precision(reason="float32r operands for full-rate PE"):
            with ExitStack() as ctx:
                _emit(nc, tc, ctx)
    nc.compile()
    _CACHE["nc"] = nc
    return nc


def _host_consts():
    if "consts" in _CACHE:
        return _CACHE["consts"]
    # RoPE tables, computed in float32 like the reference
    inv_freq = (1.0 / (ROPE_BASE ** (np.arange(0, HD, 2, dtype=np.float32) / HD))
                ).astype(np.float32)
    t = np.arange(S, dtype=np.float32)
    freqs = np.outer(t, inv_freq).astype(np.float32)          # [S, 64]
    emb = np.concatenate([freqs, freqs], axis=-1)             # [S, HD]
    cos_t = np.ascontiguousarray(np.cos(emb).T.astype(np.float32))  # [HD, S]
    sin_t = np.sin(emb).T.astype(np.float32)
    sinm_t = np.ascontiguousarray(
        np.concatenate([-sin_t[:64], sin_t[64:]], axis=0))
    # causal masks for the 4 diagonal 128-chunk offsets within a 512 q-tile
    p = np.arange(128)[:, None]
    j = np.arange(NT)[None, :]
    masks = np.stack([(m * 128 + p <= j) for m in range(4)]).astype(np.float16)
    ident = np.eye(128, dtype=np.float32)
    onec = np.ones((128, 128), np.float32)
    _CACHE["consts"] = (cos_t, sinm_t, masks, ident, onec)
    return _CACHE["consts"]


def make_in_maps(x, wq, wk, wv, wo):
    cos_t, sinm_t, masks, ident, onec = _host_consts()
    x = np.asarray(x, dtype=np.float32)
    scale = np.float32(1.0 / np.sqrt(HD))
    wq32 = np.asarray(wq, dtype=np.float32) * scale
    wk32 = np.asarray(wk, dtype=np.float32)
    wv32 = np.asarray(wv, dtype=np.float32)
    wo32 = np.asarray(wo, dtype=np.float32)
    xT32 = [np.ascontiguousarray(x[b].T) for b in range(B)]
    in_maps = []
    for d in range(NCORES):
        g, b = d // 2, d % 2
        in_maps.append({
            "xT": xT32[b],
            "wq": np.ascontiguousarray(wq32[:, g * NQH * HD:(g + 1) * NQH * HD]),
            "wk": np.ascontiguousarray(wk32[:, g * HD:(g + 1) * HD]),
            "wv": np.ascontiguousarray(wv32[:, g * HD:(g + 1) * HD]),
            "wo": np.ascontiguousarray(wo32[g * NQH * HD:(g + 1) * NQH * HD, :]),
            "cos": cos_t, "sinm": sinm_t, "masks": masks, "ident": ident,
            "onec": onec,
        })
    return in_maps


def kernel(x, wq, wk, wv, wo, attn_mask):
    nc = _build()
    in_maps = make_in_maps(x, wq, wk, wv, wo)
    res = run_bass_kernel_spmd(nc, in_maps, list(range(NCORES)))
    out = np.empty((B, S, E), np.float32)
    for b in range(B):
        o = res.results[b]["out"].astype(np.float64)
        for g in range(1, KVH):
            o += res.results[2 * g + b]["out"]
        out[b] = o.astype(np.float32)
    return out
